# revision 4
# baseline (speedup 1.0000x reference)
import sys

sys.path.insert(0, "/opt/trn_rl_repo")

import numpy as np
import concourse.bacc as bacc
import concourse.tile as tile
from concourse import mybir
from concourse.bass_utils import run_bass_kernel_spmd

F32 = mybir.dt.float32
BF16 = mybir.dt.bfloat16
AF = mybir.ActivationFunctionType
ALU = mybir.AluOpType

TS = 0.1
TRACK = 100
CA3SIGMA = 5.0
CUE_LOC = 10
BS = 256
N = 1024          # ecnum == ca1num == ca3num
NCORES = 8
BL = BS // NCORES  # 32 batch rows per core
NCH = N // 128     # 8 feature chunks
GRP = 4            # jc chunks per psum/evac group
NGRP = NCH // GRP  # 2 groups

WDT = BF16         # weight dtype for the two recurrent matmuls


def _ca3_all_np():
    centers = np.linspace(-10.0, TRACK + 10.0, N).astype(np.float32)
    x = np.arange(TRACK, dtype=np.float32)[:, None]
    return np.exp(-((centers[None, :] - x) ** 2) / (CA3SIGMA ** 2 * 2.0)).astype(np.float32)


def _fm(x):
    # (BL, N) batch-major -> feature-major [128, NCH*BL]; fm[p, c*BL+b] = x[b, c*128+p]
    return np.ascontiguousarray(
        x.reshape(BL, NCH, 128).transpose(2, 1, 0).reshape(128, NCH * BL))


def _fm_inv(a):
    # [128, NCH*BL] -> (BL, N)
    return np.ascontiguousarray(
        a.reshape(128, NCH, BL).transpose(2, 1, 0).reshape(BL, N))


def _wblocks(w):
    # (N, N) -> [128, NCH*NCH*128]; blk[p, kc*N + jc*128 + jj] = w[kc*128+p, jc*128+jj]
    return np.ascontiguousarray(
        w.reshape(NCH, 128, NCH, 128).transpose(1, 0, 2, 3).reshape(128, NCH * N))


def build():
    nc = bacc.Bacc("TRN2", target_bir_lowering=False, debug=False)
    DT = nc.dram_tensor
    i_w1 = DT("w1b", [128, NCH * N], WDT, kind="ExternalInput").ap()
    i_w2 = DT("w2b", [128, NCH * N], WDT, kind="ExternalInput").ap()
    i_wca3 = DT("wca3b", [128, NCH * N], F32, kind="ExternalInput").ap()
    i_aT = DT("aT", [128, NCH * TRACK], F32, kind="ExternalInput").ap()
    i_wact = DT("wactfm", [128, NCH * 2], F32, kind="ExternalInput").ap()
    i_bias = DT("biasfm", [128, NCH], F32, kind="ExternalInput").ap()
    i_ec3 = DT("ec3fm", [128, NCH * BL], F32, kind="ExternalInput").ap()
    i_ec5 = DT("ec5fm", [128, NCH * BL], F32, kind="ExternalInput").ap()
    i_cueA = DT("cueAfm", [128, NCH * BL], F32, kind="ExternalInput").ap()
    i_cueB = DT("cueBfm", [128, NCH * BL], F32, kind="ExternalInput").ap()
    i_noise = DT("noisefm", [TRACK, 128, NCH * BL], F32, kind="ExternalInput").ap()

    o_act = DT("o_act", [BL, 2], F32, kind="ExternalOutput").ap()
    o_ec3 = DT("o_ec3", [128, NCH * BL], F32, kind="ExternalOutput").ap()
    o_ec5 = DT("o_ec5", [128, NCH * BL], F32, kind="ExternalOutput").ap()
    o_ca1 = DT("o_ca1", [128, NCH * BL], F32, kind="ExternalOutput").ap()
    o_hisE = DT("o_hisE", [128, TRACK * NCH], F32, kind="ExternalOutput").ap()
    o_his5 = DT("o_his5", [128, TRACK * NCH], F32, kind="ExternalOutput").ap()
    o_hisC = DT("o_hisC", [128, TRACK * NCH], F32, kind="ExternalOutput").ap()

    with tile.TileContext(nc) as tc:
        with tc.tile_pool(name="wpool", bufs=1) as wpool, \
             tc.tile_pool(name="state", bufs=1) as state, \
             tc.tile_pool(name="his", bufs=1) as hisp, \
             tc.tile_pool(name="noise", bufs=3) as npool, \
             tc.tile_pool(name="ps1", bufs=2, space="PSUM") as ps1p, \
             tc.tile_pool(name="ps2", bufs=2, space="PSUM") as ps2p, \
             tc.tile_pool(name="psx", bufs=1, space="PSUM") as psxp:

            SW = NCH * BL  # 256 state width

            # ---- persistent tiles ----
            w1 = wpool.tile([128, NCH * N], WDT, tag="w1")
            w2 = wpool.tile([128, NCH * N], WDT, tag="w2")
            wca3 = wpool.tile([128, NCH * N], F32, tag="wca3")
            aT = wpool.tile([128, NCH * TRACK], F32, tag="aT")
            wact = wpool.tile([128, NCH * 2], F32, tag="wact")
            biasfm = wpool.tile([128, NCH], F32, tag="biasfm")
            cueA = wpool.tile([128, SW], F32, tag="cueA")
            cueB = wpool.tile([128, SW], F32, tag="cueB")
            projT = wpool.tile([128, NCH * TRACK], F32, tag="projT")
            bneg12 = wpool.tile([128, 1], F32, tag="bneg12")

            ec3 = state.tile([128, SW], F32, tag="ec3")
            ec5 = state.tile([128, SW], F32, tag="ec5")
            expose = state.tile([128, SW], F32, tag="expose")
            ca1 = state.tile([128, SW], F32, tag="ca1")
            sig = state.tile([128, SW], F32, tag="sig")
            q = state.tile([128, SW], F32, tag="q")
            s5 = state.tile([128, SW], F32, tag="s5")
            if WDT != F32:
                exposeb = state.tile([128, SW], WDT, tag="exposeb")
                ca1b = state.tile([128, SW], WDT, tag="ca1b")
            else:
                exposeb, ca1b = expose, ca1

            hisE = hisp.tile([128, TRACK * NCH], F32, tag="hisE")
            his5 = hisp.tile([128, TRACK * NCH], F32, tag="his5")
            hisC = hisp.tile([128, TRACK * NCH], F32, tag="hisC")

            # ---- load everything ----
            nc.sync.dma_start(w1[:], i_w1)
            nc.sync.dma_start(w2[:], i_w2)
            nc.sync.dma_start(wca3[:], i_wca3)
            nc.sync.dma_start(aT[:], i_aT)
            nc.sync.dma_start(wact[:], i_wact)
            nc.sync.dma_start(biasfm[:], i_bias)
            nc.sync.dma_start(ec3[:], i_ec3)
            nc.sync.dma_start(expose[:], i_ec3)   # ec3_expose init = ec3_last
            nc.sync.dma_start(ec5[:], i_ec5)
            nc.sync.dma_start(cueA[:], i_cueA)
            nc.sync.dma_start(cueB[:], i_cueB)
            nc.gpsimd.memset(bneg12[:], -1.2)
            if WDT != F32:
                nc.vector.tensor_copy(exposeb[:], ec3[:])

            # ---- ca3proj on device: projT[p, jc*T+t] = proj[t, jc*128+p] ----
            for jc in range(NCH):
                pp = psxp.tile([128, TRACK], F32, tag="psproj")
                for kc in range(NCH):
                    nc.tensor.matmul(
                        pp[:],
                        lhsT=wca3[:, kc * N + jc * 128: kc * N + (jc + 1) * 128],
                        rhs=aT[:, kc * TRACK:(kc + 1) * TRACK],
                        start=(kc == 0), stop=(kc == NCH - 1))
                nc.scalar.copy(projT[:, jc * TRACK:(jc + 1) * TRACK], pp[:])

            def proj_b(t, g):
                # proj broadcast view for group g at step t: [128, GRP, BL]
                sl = projT[:, jc0(g) * TRACK + t: (jc0(g) + GRP - 1) * TRACK + t + 1: TRACK]
                return sl.unsqueeze(2).to_broadcast((128, GRP, BL))

            def jc0(g):
                return g * GRP

            def gview(tile_, g):
                return tile_[:, g * GRP * BL:(g + 1) * GRP * BL]

            def gview3(tile_, g):
                return tile_[:, g * GRP * BL:(g + 1) * GRP * BL].rearrange(
                    "p (c b) -> p c b", c=GRP)

            bias_b = biasfm[:].unsqueeze(2).to_broadcast((128, NCH, BL))

            # ---- the scan ----
            for t in range(TRACK):
                nt = npool.tile([128, SW], F32, tag="noise")
                nc.sync.dma_start(nt[:], i_noise[t])

                # MM1: ca1preT = W1^T-ish blocks; jc-outer, kc-inner
                pst1 = []
                for g in range(NGRP):
                    ps = ps1p.tile([128, GRP * BL], F32, tag="ps1")
                    pst1.append(ps)
                    for j in range(GRP):
                        jc = jc0(g) + j
                        for kc in range(NCH):
                            nc.tensor.matmul(
                                ps[:, j * BL:(j + 1) * BL],
                                lhsT=w1[:, kc * N + jc * 128: kc * N + (jc + 1) * 128],
                                rhs=exposeb[:, kc * BL:(kc + 1) * BL],
                                start=(kc == 0), stop=(kc == NCH - 1))

                # MM2 emitted right after MM1 on PE; evac of MM1 groups runs
                # on ACT/DVE underneath. kc-outer so early chunks unblock it.
                # But first emit evac ops for MM1 groups (ACT/DVE engines).
                for g in range(NGRP):
                    sg = gview(sig, g)
                    nc.scalar.activation(sg, pst1[g][:], AF.Sigmoid)
                    # q = (sig + 1) * proj
                    nc.vector.scalar_tensor_tensor(
                        gview3(q, g), in0=gview3(sig, g), scalar=1.0,
                        in1=proj_b(t, g), op0=ALU.add, op1=ALU.mult)
                    # ca1 = max(q - bias, 0)
                    nc.vector.tensor_tensor(
                        gview3(ca1, g), gview3(q, g),
                        bias_b[:, jc0(g):jc0(g) + GRP, :], op=ALU.subtract)
                    nc.vector.tensor_scalar_max(gview(ca1, g), gview(ca1, g), 0.0)
                    if WDT != F32:
                        nc.vector.tensor_copy(gview(ca1b, g), gview(ca1, g))

                # MM2: ec5deltaT; jc-outer (contiguous accumulation per
                # psum slice -- interleaved slice accumulation in one bank
                # breaks has_written semantics on HW)
                pst2 = [ps2p.tile([128, GRP * BL], F32, tag="ps2", name=f"ps2_{t}_{g2}")
                        for g2 in range(NGRP)]
                for g in range(NGRP):
                    for j in range(GRP):
                        jc = jc0(g) + j
                        for kc in range(NCH):
                            nc.tensor.matmul(
                                pst2[g][:, j * BL:(j + 1) * BL],
                                lhsT=w2[:, kc * N + jc * 128: kc * N + (jc + 1) * 128],
                                rhs=ca1b[:, kc * BL:(kc + 1) * BL],
                                start=(kc == 0), stop=(kc == NCH - 1))

                for g in range(NGRP):
                    # s5 = ec5 + delta ; ec5 = 0.3*Sigmoid(4*s5-1.2)+0.7
                    nc.vector.tensor_tensor(gview(s5, g), gview(ec5, g), pst2[g][:], op=ALU.add)
                    nc.scalar.activation(gview(ec5, g), gview(s5, g), AF.Sigmoid,
                                         bias=bneg12[:], scale=4.0)
                    nc.vector.tensor_scalar(gview(ec5, g), gview(ec5, g), 0.3, 0.7,
                                            op0=ALU.mult, op1=ALU.add)
                    # ec3 *= ec5 ; expose = ec3 + noise
                    nc.vector.tensor_tensor(gview(ec3, g), gview(ec3, g), gview(ec5, g), op=ALU.mult)
                    nc.vector.tensor_tensor(gview(expose, g), gview(ec3, g), gview(nt, g), op=ALU.add)
                    if WDT != F32:
                        nc.vector.tensor_copy(gview(exposeb, g), gview(expose, g))
                    if t == CUE_LOC:
                        nc.vector.tensor_tensor(gview(ec3, g), gview(ec3, g), gview(cueA, g), op=ALU.mult)
                        nc.vector.tensor_tensor(gview(ec3, g), gview(ec3, g), gview(cueB, g), op=ALU.add)

                # history (batch row 0) on gpsimd
                nc.gpsimd.tensor_copy(hisE[:, t * NCH:(t + 1) * NCH], expose[:, 0::BL])
                nc.gpsimd.tensor_copy(his5[:, t * NCH:(t + 1) * NCH], ec5[:, 0::BL])
                nc.gpsimd.tensor_copy(hisC[:, t * NCH:(t + 1) * NCH], ca1[:, 0::BL])

            # ---- actCell = ca1 @ wca1act ----
            psa = psxp.tile([BL, 2], F32, tag="psact")
            for kc in range(NCH):
                nc.tensor.matmul(
                    psa[:],
                    lhsT=ca1[:, kc * BL:(kc + 1) * BL],
                    rhs=wact[:, kc * 2:(kc + 1) * 2],
                    start=(kc == 0), stop=(kc == NCH - 1))
            acttile = state.tile([BL, 2], F32, tag="acttile")
            nc.vector.tensor_copy(acttile[:], psa[:])

            # ---- outputs ----
            nc.sync.dma_start(o_act, acttile[:])
            nc.sync.dma_start(o_ec3, ec3[:])
            nc.sync.dma_start(o_ec5, ec5[:])
            nc.sync.dma_start(o_ca1, ca1[:])
            nc.sync.dma_start(o_hisE, hisE[:])
            nc.sync.dma_start(o_his5, his5[:])
            nc.sync.dma_start(o_hisC, hisC[:])

    nc.finalize()
    return nc


_NC_CACHE = None


def _get_nc():
    global _NC_CACHE
    if _NC_CACHE is None:
        _NC_CACHE = build()
    return _NC_CACHE


def _cast_w(w):
    if WDT == BF16:
        import jax.numpy as jnp
        return np.asarray(jnp.asarray(w, dtype=jnp.bfloat16))
    return w.astype(np.float32)


def kernel(cue, ec3_last, ec5_last, ca1_last, pink_noise,
           ca1bias, wca3ca1, wec3ca1, wca1ec5, wca1act, actbias):
    cue = np.asarray(cue, np.int32)
    ec3_last = np.asarray(ec3_last, np.float32)
    ec5_last = np.asarray(ec5_last, np.float32)
    pink_noise = np.asarray(pink_noise, np.float32)
    ca1bias = np.asarray(ca1bias, np.float32)
    wca3ca1 = np.asarray(wca3ca1, np.float32)
    wec3ca1 = np.asarray(wec3ca1, np.float32)
    wca1ec5 = np.asarray(wca1ec5, np.float32)
    wca1act = np.asarray(wca1act, np.float32)
    actbias = np.asarray(actbias, np.float32)

    nc = _get_nc()

    w1b = _cast_w(_wblocks(wec3ca1))
    w2b = _cast_w(_wblocks(wca1ec5))
    wca3b = _wblocks(wca3ca1)
    A = _ca3_all_np()                      # (100, N)
    aT = np.ascontiguousarray(             # [128, NCH*100]
        A.reshape(TRACK, NCH, 128).transpose(2, 1, 0).reshape(128, NCH * TRACK))
    wactfm = np.ascontiguousarray(
        wca1act.reshape(NCH, 128, 2).transpose(1, 0, 2).reshape(128, NCH * 2))
    biasfm = np.ascontiguousarray(ca1bias.reshape(NCH, 128).T)

    shared = dict(w1b=w1b, w2b=w2b, wca3b=wca3b, aT=aT, wactfm=wactfm, biasfm=biasfm)

    in_maps = []
    for c in range(NCORES):
        sl = slice(c * BL, (c + 1) * BL)
        cue_f = cue[sl].astype(np.float32)
        m = dict(shared)
        m["ec3fm"] = _fm(ec3_last[sl])
        m["ec5fm"] = _fm(ec5_last[sl])
        m["cueAfm"] = _fm(1.0 - 0.6 * cue_f)
        m["cueBfm"] = _fm(0.6 * cue_f)
        pn = pink_noise[:, sl, :]          # (100, BL, N)
        m["noisefm"] = np.ascontiguousarray(
            pn.reshape(TRACK, BL, NCH, 128).transpose(0, 3, 2, 1).reshape(TRACK, 128, NCH * BL))
        in_maps.append(m)

    import kernel as _self
    _self.LAST_IN_MAPS = in_maps
    res = run_bass_kernel_spmd(nc, in_maps, core_ids=list(range(NCORES)))
    rs = res.results

    actCell = np.concatenate([rs[c]["o_act"] for c in range(NCORES)], 0) + actbias[None, :]
    ec3 = np.concatenate([_fm_inv(rs[c]["o_ec3"]) for c in range(NCORES)], 0)
    ec5 = np.concatenate([_fm_inv(rs[c]["o_ec5"]) for c in range(NCORES)], 0)
    ca1 = np.concatenate([_fm_inv(rs[c]["o_ca1"]) for c in range(NCORES)], 0)

    def his(name):
        b = rs[0][name]                    # [128, TRACK*NCH]
        return np.ascontiguousarray(
            b.reshape(128, TRACK, NCH).transpose(1, 2, 0).reshape(TRACK, N))

    return (actCell, his("o_hisE"), his("o_his5"), his("o_hisC"), ec3, ec5, ca1)
